# revision 1
# baseline (speedup 1.0000x reference)
"""Fused single-pass multi-head causal attention on 8 Trainium2 NeuronCores.

Sharding: 8 cores = 4 batches x 2 head-halves.  Each core computes, for one
batch, 8 of the 16 heads end-to-end (QKV projection with column-sharded
weights, causal attention, and a partial output projection with row-sharded
Wo).  The host sums the two partial outputs per batch and adds bo.

The causal kernel is a fused single pass over the sequence in 512-row steps
r=0..3 (see _build_fused_kernel below).  Causality makes the single pass
legal: queries in block r only attend to keys/values in rows <= (r+1)*512,
which are projected in steps <= r.  All matmuls are bf16 with fp32 PSUM
accumulation (max rel err vs the fp32 reference ~4e-3).

Non-causal masks (dense/general) fall back to a three-phase kernel.
"""

import sys

if "/opt/trn_rl_repo" not in sys.path:
    sys.path.insert(0, "/opt/trn_rl_repo")

import numpy as np

import concourse.bass as bass  # noqa: F401
import concourse.tile as tile
from concourse import bacc, mybir

F32 = mybir.dt.float32
BF16 = mybir.dt.bfloat16

# Problem shape (hardcoded per the harness contract).
B, S, D, H = 4, 2048, 1024, 16
DH = D // H
N_CORES = 8
HC = H // 2              # heads per core
DC = HC * DH             # feature columns per core (512)
QB = 512                 # query block
KB = 128                 # key block
G_KB = 2                 # key blocks per exp group

N_KC = D // 128          # contraction chunks (8)
N_CH = DC // 128         # feature chunks per core (4); 2 heads per chunk
N_RB = S // QB           # sequence steps (4)
N_KB = S // KB           # key blocks (16)
KB_PER_QB = QB // KB     # 4


def _build_fused_kernel(mm_dt=BF16):
    nc = bacc.Bacc("TRN2", target_bir_lowering=False, debug=False)

    xq_d = nc.dram_tensor("xqT", [D, S], mm_dt, kind="ExternalInput")
    xk_d = nc.dram_tensor("xkT", [D, S], mm_dt, kind="ExternalInput")
    xv_d = nc.dram_tensor("xvT", [D, S], mm_dt, kind="ExternalInput")
    wq_d = nc.dram_tensor("wq", [D, DC], mm_dt, kind="ExternalInput")
    wk_d = nc.dram_tensor("wk", [D, DC], mm_dt, kind="ExternalInput")
    wv_d = nc.dram_tensor("wv", [D, DC], mm_dt, kind="ExternalInput")
    wo_d = nc.dram_tensor("wo", [DC, D], mm_dt, kind="ExternalInput")
    out_d = nc.dram_tensor("out", [S, D], F32, kind="ExternalOutput")

    inv_sqrt_dh = 1.0 / float(np.sqrt(DH))

    xq_r = xq_d.ap().rearrange("(c p) s -> p c s", p=128)
    xk_r = xk_d.ap().rearrange("(c p) s -> p c s", p=128)
    xv_r = xv_d.ap().rearrange("(c p) s -> p c s", p=128)
    wq_r = wq_d.ap().rearrange("(c p) n -> p c n", p=128)
    wk_r = wk_d.ap().rearrange("(c p) n -> p c n", p=128)
    wv_r = wv_d.ap().rearrange("(c p) n -> p c n", p=128)

    with tile.TileContext(nc) as tc:
        with (
            tc.tile_pool(name="res", bufs=1) as res,
            tc.tile_pool(name="small", bufs=1) as small,
            tc.tile_pool(name="xs", bufs=2) as xs,
            tc.tile_pool(name="bex", bufs=2) as bex,
            tc.tile_pool(name="cb", bufs=8) as cbp,
            tc.tile_pool(name="seb", bufs=4) as sebp,
            tc.tile_pool(name="ot", bufs=2) as otp,
            tc.tile_pool(name="sco", bufs=1, space="PSUM") as sco,
            tc.tile_pool(name="pctx", bufs=1, space="PSUM") as pctx,
            tc.tile_pool(name="aux", bufs=2, space="PSUM") as aux,
        ):
            tri = small.tile([KB, KB], F32, tag="tri")
            nc.gpsimd.memset(tri[:], 0.0)
            nc.gpsimd.affine_select(
                out=tri[:], in_=tri[:],
                compare_op=mybir.AluOpType.is_ge,
                fill=-1e9, base=0,
                pattern=[[1, KB]], channel_multiplier=-1,
            )
            ones_c = small.tile([128, HC], mm_dt, tag="ones_c")
            nc.gpsimd.memset(ones_c[:], 1.0)

            qT = [[res.tile([128, QB], mm_dt, tag=f"qT{c}r{r}",
                            name=f"qT{c}r{r}")
                   for r in range(N_RB)] for c in range(N_CH)]
            kT = [[res.tile([128, QB], mm_dt, tag=f"kT{c}r{r}",
                            name=f"kT{c}r{r}")
                   for r in range(N_RB)] for c in range(N_CH)]
            cxT = [[res.tile([128, QB], mm_dt, tag=f"cx{c}r{r}",
                             name=f"cx{c}r{r}")
                    for r in range(N_RB)] for c in range(N_CH)]
            v_aug = [res.tile([128, HC, DH + 1], mm_dt, tag=f"v{b}",
                              name=f"v{b}")
                     for b in range(N_KB)]
            wv_t = res.tile([128, N_KC, DC], mm_dt, tag="wv")
            wq_t = res.tile([128, N_KC, DC], mm_dt, tag="wq")
            wk_t = res.tile([128, N_KC, DC], mm_dt, tag="wk")
            wo_t = res.tile([128, N_CH, D], mm_dt, tag="wo")

            # ones columns of v_aug never change: write them once up front
            # (DVE is idle while the first DMAs land)
            for b in range(N_KB):
                nc.vector.tensor_copy(v_aug[b][:, :, DH], ones_c[:])

            rings = [nc.sync.dma_start, nc.scalar.dma_start]
            ring_i = [0]

            def dma(dst, src):
                rings[ring_i[0]](dst, src)
                ring_i[0] ^= 1

            x_tiles = {}

            def load_x(r):
                xv_t = xs.tile([128, N_KC, QB], mm_dt, tag="xv")
                nc.sync.dma_start(xv_t[:], xv_r[:, :, r * QB:(r + 1) * QB])
                xq_t = xs.tile([128, N_KC, QB], mm_dt, tag="xq")
                nc.sync.dma_start(xq_t[:], xq_r[:, :, r * QB:(r + 1) * QB])
                xk_t = xs.tile([128, N_KC, QB], mm_dt, tag="xk")
                nc.sync.dma_start(xk_t[:], xk_r[:, :, r * QB:(r + 1) * QB])
                x_tiles[r] = (xv_t, xq_t, xk_t)

            # initial DMAs: q-projection data first, then k, then v, then wo
            xq_t0 = xs.tile([128, N_KC, QB], mm_dt, tag="xq")
            xk_t0 = xs.tile([128, N_KC, QB], mm_dt, tag="xk")
            xv_t0 = xs.tile([128, N_KC, QB], mm_dt, tag="xv")
            dma(wq_t[:, 0:4, :], wq_r[:, 0:4, :])
            dma(xq_t0[:, 0:4, :], xq_r[:, 0:4, 0:QB])
            dma(wq_t[:, 4:, :], wq_r[:, 4:, :])
            dma(xq_t0[:, 4:, :], xq_r[:, 4:, 0:QB])
            dma(wk_t[:], wk_r)
            dma(xk_t0[:], xk_r[:, :, 0:QB])
            dma(wv_t[:], wv_r)
            dma(xv_t0[:], xv_r[:, :, 0:QB])
            x_tiles[0] = (xv_t0, xq_t0, xk_t0)
            dma(wo_t[:], wo_d.ap().rearrange("(c p) n -> p c n", p=128))

            def v_thunks(r):
                xv_t = x_tiles[r][0]
                out = []
                for rr in range(KB_PER_QB):
                    def th(rr=rr, xv_t=xv_t):
                        b = KB_PER_QB * r + rr
                        ps = aux.tile([128, DC], F32, tag="aux", name="ps")
                        for kc in range(N_KC):
                            nc.tensor.matmul(
                                ps[:], xv_t[:, kc, rr * KB:(rr + 1) * KB],
                                wv_t[:, kc, :],
                                start=(kc == 0), stop=(kc == N_KC - 1))
                        nc.scalar.activation(
                            v_aug[b][:, :, 0:DH],
                            ps[:].rearrange("p (h d) -> p h d", h=HC),
                            mybir.ActivationFunctionType.Copy)
                    out.append(th)
                return out

            def q_thunks(r):
                xq_t = x_tiles[r][1]
                out = []
                for c in range(N_CH):
                    def th(c=c, xq_t=xq_t):
                        psq = aux.tile([128, QB], F32, tag="aux", name="ps")
                        for kc in range(N_KC):
                            nc.tensor.matmul(
                                psq[:], wq_t[:, kc, c * 128:(c + 1) * 128],
                                xq_t[:, kc, :],
                                start=(kc == 0), stop=(kc == N_KC - 1))
                        nc.scalar.activation(
                            qT[c][r][:], psq[:],
                            mybir.ActivationFunctionType.Copy)
                    out.append(th)
                return out

            def k_thunks(r):
                xk_t = x_tiles[r][2]
                out = []
                for c in range(N_CH):
                    def th(c=c, xk_t=xk_t):
                        psk = aux.tile([128, QB], F32, tag="aux", name="ps")
                        for kc in range(N_KC):
                            nc.tensor.matmul(
                                psk[:], wk_t[:, kc, c * 128:(c + 1) * 128],
                                xk_t[:, kc, :],
                                start=(kc == 0), stop=(kc == N_KC - 1))
                        nc.scalar.activation(
                            kT[c][r][:], psk[:],
                            mybir.ActivationFunctionType.Copy)
                    out.append(th)
                return out

            def out_thunks(r):
                out = []
                for rr in range(KB_PER_QB):
                    def th(rr=rr):
                        q0 = r * QB + rr * KB
                        pos = [aux.tile([128, 512], F32, tag="aux",
                                        name="pos") for _ in range(2)]
                        for c in range(N_CH):
                            for oc in range(2):
                                nc.tensor.matmul(
                                    pos[oc][:],
                                    cxT[c][r][:, rr * KB:(rr + 1) * KB],
                                    wo_t[:, c, oc * 512:(oc + 1) * 512],
                                    start=(c == 0), stop=(c == N_CH - 1))
                        ot = otp.tile([128, D], F32, tag="ot")
                        for oc in range(2):
                            nc.scalar.activation(
                                ot[:, oc * 512:(oc + 1) * 512], pos[oc][:],
                                mybir.ActivationFunctionType.Copy)
                        nc.sync.dma_start(out_d.ap()[q0:q0 + KB, :], ot[:])
                    out.append(th)
                return out

            def kT_slice(c, b0, kb):
                return kT[c][kb // KB_PER_QB][
                    b0:b0 + 64, (kb % KB_PER_QB) * KB:
                                (kb % KB_PER_QB + 1) * KB]

            def attention(r, filler, delay=0):
                q0 = r * QB
                kmax = KB_PER_QB * (r + 1)
                n_g = kmax // G_KB
                total_groups = n_g * N_CH
                gi = 0
                fi = 0
                norm_jobs = []

                def pace():
                    nonlocal fi
                    if gi <= delay:
                        return
                    want = int(len(filler) * (gi - delay)
                               / max(1, total_groups - delay))
                    while fi < min(want, len(filler)):
                        filler[fi]()
                        fi += 1

                for c in range(N_CH):
                    heads = [{"h": 2 * c + half, "base": half * 64,
                              "tag": half} for half in range(2)]
                    for hd in heads:
                        hd["psc"] = pctx.tile(
                            [DH + 1, QB], F32, tag=f"pctx{hd['tag']}",
                            name=f"psc{hd['tag']}")
                    prev = None
                    for g in range(n_g):
                        kbs = list(range(G_KB * g, G_KB * (g + 1)))
                        for hd in heads:
                            hd["pss"] = sco.tile(
                                [128, G_KB * QB], F32, tag=f"s{hd['tag']}",
                                name=f"pss{hd['tag']}")
                            hd["ext"] = bex.tile(
                                [128, G_KB * QB], mm_dt,
                                tag=f"e{hd['tag']}", name=f"ext{hd['tag']}")
                        for i, kb in enumerate(kbs):
                            j = kb - r * KB_PER_QB
                            off = j * KB if j > 0 else 0
                            for hd in heads:
                                b0 = hd["base"]
                                nc.tensor.matmul(
                                    hd["pss"][:, i * QB + off:(i + 1) * QB],
                                    kT_slice(c, b0, kb),
                                    qT[c][r][b0:b0 + 64, off:QB],
                                    start=True, stop=True)
                            if j >= 0:
                                for hd in heads:
                                    nc.vector.tensor_tensor(
                                        hd["pss"][:, i * QB + off:
                                                  i * QB + off + KB],
                                        hd["pss"][:, i * QB + off:
                                                  i * QB + off + KB],
                                        tri[:], op=mybir.AluOpType.add)
                        if prev is not None:
                            pkbs, pexts = prev
                            for i, kb in enumerate(pkbs):
                                j = kb - r * KB_PER_QB
                                off = j * KB if j > 0 else 0
                                for hd in heads:
                                    nc.tensor.matmul(
                                        hd["psc"][:, off:QB],
                                        v_aug[kb][:, hd["h"], :],
                                        pexts[hd["tag"]][:, i * QB + off:
                                                         (i + 1) * QB],
                                        start=(kb == 0),
                                        stop=(kb == kmax - 1))
                        whole = all(kb - r * KB_PER_QB < 0 for kb in kbs)
                        for hd in heads:
                            if whole:
                                nc.scalar.activation(
                                    hd["ext"][:], hd["pss"][:],
                                    mybir.ActivationFunctionType.Exp,
                                    scale=inv_sqrt_dh)
                            else:
                                for i, kb in enumerate(kbs):
                                    j = kb - r * KB_PER_QB
                                    off = j * KB if j > 0 else 0
                                    nc.scalar.activation(
                                        hd["ext"][:, i * QB + off:
                                                  (i + 1) * QB],
                                        hd["pss"][:, i * QB + off:
                                                  (i + 1) * QB],
                                        mybir.ActivationFunctionType.Exp,
                                        scale=inv_sqrt_dh)
                        gi += 1
                        pace()
                        prev = (kbs, {hd["tag"]: hd["ext"]
                                      for hd in heads})
                    pkbs, pexts = prev
                    for i, kb in enumerate(pkbs):
                        j = kb - r * KB_PER_QB
                        off = j * KB if j > 0 else 0
                        for hd in heads:
                            nc.tensor.matmul(
                                hd["psc"][:, off:QB],
                                v_aug[kb][:, hd["h"], :],
                                pexts[hd["tag"]][:, i * QB + off:
                                                 (i + 1) * QB],
                                start=(kb == 0), stop=(kb == kmax - 1))
                    for hd in heads:
                        cb = cbp.tile([DH + 1, QB], F32, tag="cb")
                        nc.vector.tensor_copy(cb[:], hd["psc"][:])
                        norm_jobs.append((c, hd["base"], cb))
                    if r == N_RB - 1:
                        # last step: normalize each chunk immediately so the
                        # final output projection isn't gated at the end
                        for cc, b0, cb in norm_jobs:
                            _normalize(cc, r, b0, cb)
                        norm_jobs = []
                while fi < len(filler):
                    filler[fi]()
                    fi += 1
                for cc, b0, cb in norm_jobs:
                    _normalize(cc, r, b0, cb)

            def _normalize(cc, r, b0, cb):
                se_r = sebp.tile([1, QB], F32, tag="ser")
                nc.vector.reciprocal(se_r[:], cb[DH:DH + 1, :])
                se_b = sebp.tile([64, QB], F32, tag="seb")
                nc.gpsimd.partition_broadcast(se_b[:], se_r[:])
                nc.vector.tensor_tensor(
                    cxT[cc][r][b0:b0 + 64, :],
                    cb[0:DH, :], se_b[:],
                    op=mybir.AluOpType.mult)

            # ---- the fused pass -----------------------------------------
            load_x(1)
            for th in q_thunks(0):
                th()
            for th in k_thunks(0):
                th()
            for th in v_thunks(0):
                th()
            attention(0, v_thunks(1) + q_thunks(1) + k_thunks(1))
            # filler rebalance: att(3) has the most ACT-bound slack but
            # no next-step projections, so it gets BOTH out(1) and out(2);
            # att(2) keeps only proj(3)
            load_x(2)
            attention(1, v_thunks(2) + q_thunks(2) + k_thunks(2)
                      + out_thunks(0))
            load_x(3)
            attention(2, v_thunks(3) + q_thunks(3) + k_thunks(3))
            attention(3, out_thunks(1) + out_thunks(2), delay=2)
            for th in out_thunks(N_RB - 1):
                th()

    nc.compile()
    return nc


# ---------------------------------------------------------------------------
# Fallback three-phase kernel for non-causal masks (dense / general).
# ---------------------------------------------------------------------------

def _build_fallback_kernel(mask_mode, mm_dt=BF16):
    RB = 512
    nc = bacc.Bacc("TRN2", target_bir_lowering=False, debug=False)

    xq_d = nc.dram_tensor("xqT", [D, S], mm_dt, kind="ExternalInput")
    xk_d = nc.dram_tensor("xkT", [D, S], mm_dt, kind="ExternalInput")
    xv_d = nc.dram_tensor("xvT", [D, S], mm_dt, kind="ExternalInput")
    wq_d = nc.dram_tensor("wq", [D, DC], mm_dt, kind="ExternalInput")
    wk_d = nc.dram_tensor("wk", [D, DC], mm_dt, kind="ExternalInput")
    wv_d = nc.dram_tensor("wv", [D, DC], mm_dt, kind="ExternalInput")
    wo_d = nc.dram_tensor("wo", [DC, D], mm_dt, kind="ExternalInput")
    if mask_mode == "general":
        mneg_d = nc.dram_tensor("maskTneg", [S, S], F32, kind="ExternalInput")
    out_d = nc.dram_tensor("out", [S, D], F32, kind="ExternalOutput")

    n_kc = D // 128
    n_ch = DC // 128
    n_rb = S // RB
    n_qb = S // QB
    n_kb = S // KB

    inv_sqrt_dh = 1.0 / float(np.sqrt(DH))

    xq_r = xq_d.ap().rearrange("(c p) s -> p c s", p=128)
    xk_r = xk_d.ap().rearrange("(c p) s -> p c s", p=128)
    xv_r = xv_d.ap().rearrange("(c p) s -> p c s", p=128)

    with tile.TileContext(nc) as tc:
        with (
            tc.tile_pool(name="res", bufs=1) as res,
            tc.tile_pool(name="small", bufs=1) as small,
        ):
            ones_c = small.tile([128, HC], mm_dt, tag="ones_c")
            nc.gpsimd.memset(ones_c[:], 1.0)

            qT = [res.tile([128, S], mm_dt, tag=f"qT{c}", name=f"qT{c}")
                  for c in range(n_ch)]
            kT = [res.tile([128, S], mm_dt, tag=f"kT{c}", name=f"kT{c}")
                  for c in range(n_ch)]
            v_aug = [res.tile([128, HC, DH + 1], mm_dt, tag=f"v{r}",
                              name=f"v{r}")
                     for r in range(n_kb)]

            with (
                tc.tile_pool(name="wa", bufs=1) as wa,
                tc.tile_pool(name="xs", bufs=3) as xs,
                tc.tile_pool(name="aps", bufs=2, space="PSUM") as aps,
                tc.tile_pool(name="aqk", bufs=3, space="PSUM") as aqk,
            ):
                wv_t = wa.tile([128, n_kc, DC], mm_dt, tag="wv")
                wq_t = wa.tile([128, n_kc, DC], mm_dt, tag="wq")
                wk_t = wa.tile([128, n_kc, DC], mm_dt, tag="wk")
                nc.sync.dma_start(
                    wv_t[:], wv_d.ap().rearrange("(c p) n -> p c n", p=128))
                nc.sync.dma_start(
                    wq_t[:], wq_d.ap().rearrange("(c p) n -> p c n", p=128))
                nc.sync.dma_start(
                    wk_t[:], wk_d.ap().rearrange("(c p) n -> p c n", p=128))

                for r2 in range(n_kb // 2):
                    xv_t = xs.tile([128, n_kc, 2 * KB], mm_dt, tag="x")
                    nc.sync.dma_start(
                        xv_t[:], xv_r[:, :, 2 * r2 * KB:(2 * r2 + 2) * KB])
                    for rr in range(2):
                        r = 2 * r2 + rr
                        ps = aps.tile([128, DC], F32, tag="pv")
                        for kc in range(n_kc):
                            nc.tensor.matmul(
                                ps[:], xv_t[:, kc, rr * KB:(rr + 1) * KB],
                                wv_t[:, kc, :],
                                start=(kc == 0), stop=(kc == n_kc - 1))
                        nc.vector.tensor_copy(v_aug[r][:, :, DH], ones_c[:])
                        nc.scalar.activation(
                            v_aug[r][:, :, 0:DH],
                            ps[:].rearrange("p (h d) -> p h d", h=HC),
                            mybir.ActivationFunctionType.Copy)

                for r in range(n_rb):
                    xq_t = xs.tile([128, n_kc, RB], mm_dt, tag="x")
                    xk_t = xs.tile([128, n_kc, RB], mm_dt, tag="x")
                    nc.sync.dma_start(
                        xq_t[:], xq_r[:, :, r * RB:(r + 1) * RB])
                    nc.sync.dma_start(
                        xk_t[:], xk_r[:, :, r * RB:(r + 1) * RB])
                    for c in range(n_ch):
                        psq = aqk.tile([128, RB], F32, tag="pq")
                        psk = aqk.tile([128, RB], F32, tag="pk")
                        for kc in range(n_kc):
                            nc.tensor.matmul(
                                psq[:], wq_t[:, kc, c * 128:(c + 1) * 128],
                                xq_t[:, kc, :],
                                start=(kc == 0), stop=(kc == n_kc - 1))
                        for kc in range(n_kc):
                            nc.tensor.matmul(
                                psk[:], wk_t[:, kc, c * 128:(c + 1) * 128],
                                xk_t[:, kc, :],
                                start=(kc == 0), stop=(kc == n_kc - 1))
                        nc.scalar.activation(
                            qT[c][:, r * RB:(r + 1) * RB], psq[:],
                            mybir.ActivationFunctionType.Copy)
                        nc.vector.tensor_copy(
                            kT[c][:, r * RB:(r + 1) * RB], psk[:])

            with tc.tile_pool(name="cw", bufs=1) as cw:
              ctxT = [cw.tile([128, S], mm_dt, tag=f"ctxT{c}",
                              name=f"ctxT{c}") for c in range(n_ch)]
              with (
                tc.tile_pool(name="bex", bufs=2) as bex,
                tc.tile_pool(name="bse", bufs=4) as bse,
                tc.tile_pool(name="bps", bufs=1, space="PSUM") as bps,
                tc.tile_pool(name="bctx", bufs=2, space="PSUM") as bctx,
              ):
                wo_t = cw.tile([128, n_ch, D], mm_dt, tag="wo")
                nc.sync.dma_start(
                    wo_t[:], wo_d.ap().rearrange("(c p) n -> p c n", p=128))

                for c in range(n_ch):
                    heads = [{"h": 2 * c + half, "base": half * 64,
                              "tag": half} for half in range(2)]
                    for qb in range(n_qb):
                        q0 = qb * QB
                        for hd in heads:
                            hd["psc"] = bctx.tile(
                                [DH + 1, QB], F32, tag=f"pctx{hd['tag']}",
                                name=f"psc{hd['tag']}")
                        n_g = n_kb // G_KB
                        for g in range(n_g):
                            kbs = list(range(G_KB * g, G_KB * (g + 1)))
                            for hd in heads:
                                hd["pss"] = bps.tile(
                                    [128, G_KB * QB], F32,
                                    tag=f"s{hd['tag']}",
                                    name=f"pss{hd['tag']}")
                                hd["ext"] = bex.tile(
                                    [128, G_KB * QB], mm_dt,
                                    tag=f"e{hd['tag']}",
                                    name=f"ext{hd['tag']}")
                            for i, kb in enumerate(kbs):
                                for hd in heads:
                                    b0 = hd["base"]
                                    nc.tensor.matmul(
                                        hd["pss"][:, i * QB:(i + 1) * QB],
                                        kT[c][b0:b0 + 64,
                                              kb * KB:(kb + 1) * KB],
                                        qT[c][b0:b0 + 64, q0:q0 + QB],
                                        start=True, stop=True)
                                if mask_mode == "general":
                                    for hd in heads:
                                        mng = bse.tile([128, QB], F32,
                                                       tag="mng")
                                        nc.sync.dma_start(
                                            mng[:],
                                            mneg_d.ap()
                                            [kb * KB:(kb + 1) * KB,
                                             q0:q0 + QB])
                                        nc.vector.tensor_tensor(
                                            hd["pss"][:, i * QB:
                                                      (i + 1) * QB],
                                            hd["pss"][:, i * QB:
                                                      (i + 1) * QB],
                                            mng[:], op=mybir.AluOpType.add)
                            for hd in heads:
                                nc.scalar.activation(
                                    hd["ext"][:], hd["pss"][:],
                                    mybir.ActivationFunctionType.Exp,
                                    scale=inv_sqrt_dh)
                            for i, kb in enumerate(kbs):
                                for hd in heads:
                                    nc.tensor.matmul(
                                        hd["psc"][:],
                                        v_aug[kb][:, hd["h"], :],
                                        hd["ext"][:, i * QB:(i + 1) * QB],
                                        start=(kb == 0),
                                        stop=(kb == n_kb - 1))
                        for hd in heads:
                            b0 = hd["base"]
                            se_r = bse.tile([1, QB], F32, tag="ser")
                            se_b = bse.tile([64, QB], F32, tag="seb")
                            nc.vector.reciprocal(
                                se_r[:], hd["psc"][DH:DH + 1, :])
                            nc.gpsimd.partition_broadcast(se_b[:], se_r[:])
                            nc.vector.tensor_tensor(
                                ctxT[c][b0:b0 + 64, q0:q0 + QB],
                                hd["psc"][0:DH, :], se_b[:],
                                op=mybir.AluOpType.mult)

              with (
                  tc.tile_pool(name="cout", bufs=2) as cout,
                  tc.tile_pool(name="cps", bufs=2, space="PSUM") as cps,
              ):
                  for r2 in range(n_kb // 2):
                      ot = cout.tile([128, 2, D], F32, tag="ot")
                      for rr in range(2):
                          r = 2 * r2 + rr
                          pos = [cps.tile([128, 512], F32, tag=f"po{oc}",
                                          name=f"po{oc}")
                                 for oc in range(D // 512)]
                          for c in range(n_ch):
                              for oc in range(D // 512):
                                  nc.tensor.matmul(
                                      pos[oc][:],
                                      ctxT[c][:, r * KB:(r + 1) * KB],
                                      wo_t[:, c, oc * 512:(oc + 1) * 512],
                                      start=(c == 0), stop=(c == n_ch - 1))
                          for oc in range(D // 512):
                              nc.scalar.activation(
                                  ot[:, rr, oc * 512:(oc + 1) * 512],
                                  pos[oc][:],
                                  mybir.ActivationFunctionType.Copy)
                      nc.sync.dma_start(
                          out_d.ap().rearrange("(r2 rr p) n -> p r2 rr n",
                                               rr=2, p=128)[:, r2],
                          ot[:])

    nc.compile()
    return nc


_KERNEL_CACHE = {}


def _get_kernel(mask_mode):
    if mask_mode not in _KERNEL_CACHE:
        if mask_mode == "causal":
            _KERNEL_CACHE[mask_mode] = _build_fused_kernel()
        else:
            _KERNEL_CACHE[mask_mode] = _build_fallback_kernel(mask_mode)
    return _KERNEL_CACHE[mask_mode]


def _classify_mask(mask):
    m = np.asarray(mask).reshape(S, S)
    if not m.any():
        return "dense"
    iu = np.triu_indices(S, 1)
    causal = np.zeros((S, S), np.float32)
    causal[iu] = 1.0
    if np.array_equal(m, causal):
        return "causal"
    return "general"


def _bf16(a):
    import ml_dtypes
    return np.asarray(a).astype(ml_dtypes.bfloat16)


def make_in_maps(queries, keys, values, mask, Wq, bq, Wk, bk, Wv, bv, Wo, bo):
    mask_mode = _classify_mask(mask)
    assert not np.any(bq) and not np.any(bk) and not np.any(bv), (
        "nonzero qkv biases not supported by this kernel build")
    in_maps = []
    for core in range(N_CORES):
        b, half = divmod(core, 2)
        cols = slice(half * DC, (half + 1) * DC)
        m = {
            "xqT": _bf16(np.asarray(queries)[b].T),
            "xkT": _bf16(np.asarray(keys)[b].T),
            "xvT": _bf16(np.asarray(values)[b].T),
            "wq": _bf16(np.asarray(Wq)[:, cols]),
            "wk": _bf16(np.asarray(Wk)[:, cols]),
            "wv": _bf16(np.asarray(Wv)[:, cols]),
            "wo": _bf16(np.asarray(Wo)[half * DC:(half + 1) * DC, :]),
        }
        if mask_mode == "general":
            m["maskTneg"] = np.ascontiguousarray(
                np.asarray(mask).reshape(S, S).T * np.float32(-1e9))
        in_maps.append(m)
    return mask_mode, in_maps


def combine_results(results, bo):
    out = np.empty((B, S, D), np.float32)
    for b in range(B):
        out[b] = results[2 * b]["out"] + results[2 * b + 1]["out"]
    out += np.asarray(bo).reshape(1, 1, D).astype(np.float32)
    return out


def kernel(queries, keys, values, mask, Wq, bq, Wk, bk, Wv, bv, Wo, bo):
    from concourse import bass_utils

    mask_mode, in_maps = make_in_maps(
        queries, keys, values, mask, Wq, bq, Wk, bk, Wv, bv, Wo, bo)
    nc = _get_kernel(mask_mode)
    res = bass_utils.run_bass_kernel_spmd(
        nc, in_maps, core_ids=list(range(N_CORES)), trace=False)
    return combine_results(res.results, np.asarray(bo))

